# revision 30
# baseline (speedup 1.0000x reference)
"""MetaS4 step kernel for Trainium2, 8 NeuronCores, batch-parallel.

Model (per batch row b, channel h, state index n):
    ns_re = dA_re*s_re - dA_im*s_im + dB_re*u
    ns_im = dA_re*s_im + dA_im*s_re + dB_im*u
    y1    = 2*sum_n(C_re*ns_re - C_im*ns_im)        -- folded to P/Q form below
    y     = gelu(y1 + u*D)
    z     = y @ W.T + b ; out = z[:, :H] * sigmoid(z[:, H:])

Readout is algebraically folded so it reads the OLD state directly:
    y1 = sum_n(P*s_re + Qn*s_im) + u*R
    P  = 2*(C_re*dA_re - C_im*dA_im)
    Qn = -2*(C_re*dA_im + C_im*dA_re)
    R  = 2*sum_n(C_re*dB_re - C_im*dB_im);  Dp = D + R
so gelu input is y1_s + u*Dp, decoupling the readout from the ns chain.

Layout: batch is sharded 128 rows/core.  On-chip, channels h live on the
128 partitions (8 chunks of 128), free dims are (b, n).  All parameters
are per-partition [128, chunk] tables broadcast along free dims via
stride-0 APs, so no parameter materialization is needed.
"""

import os
import sys
from concurrent.futures import ThreadPoolExecutor

import numpy as np

for _p in ("/opt/trn_rl_repo",):
    if _p not in sys.path and os.path.isdir(_p):
        sys.path.insert(0, _p)

import concourse.bass as bass
import concourse.tile as tile
from concourse import bacc, mybir
from concourse.bass_utils import run_bass_kernel_spmd

B, H, N = 1024, 1024, 32
NCORES = 8
BC = B // NCORES          # batch rows per core = 128
CH = 8                    # h chunks of 128 partitions
BH = 2                    # b halves per chunk
BHS = BC // BH            # 64 rows per half
F32 = mybir.dt.float32
J = 2 * H                 # MLP output width

_CACHE = {}


def _build_program():
    nc = bacc.Bacc(
        "TRN2",
        target_bir_lowering=False,
        debug=False,
        enable_asserts=False,
        num_devices=NCORES,
    )

    # Per-core tensors ([H, BC, N] is state transposed so the h-partition
    # DMA is contiguous per partition).
    s_re_d = nc.dram_tensor("s_re", [H, BC, N], F32, kind="ExternalInput")
    s_im_d = nc.dram_tensor("s_im", [H, BC, N], F32, kind="ExternalInput")
    u_t_d = nc.dram_tensor("u_t", [128, CH, BC], F32, kind="ExternalInput")
    # Replicated parameter tables, host-prearranged to partition-major.
    par_d = {
        name: nc.dram_tensor(name, [128, CH, N], F32, kind="ExternalInput")
        for name in ("dA_re", "dA_im", "dB_re", "dB_im", "P", "Qn")
    }
    dp_d = nc.dram_tensor("Dp", [128, CH], F32, kind="ExternalInput")
    w_d = nc.dram_tensor("w_t", [128, CH, J], F32, kind="ExternalInput")
    bias_d = nc.dram_tensor("bias", [1, J], F32, kind="ExternalInput")

    ns_re_d = nc.dram_tensor("ns_re", [H, BC, N], F32, kind="ExternalOutput")
    ns_im_d = nc.dram_tensor("ns_im", [H, BC, N], F32, kind="ExternalOutput")
    y_out_d = nc.dram_tensor("y_out", [BC, H], F32, kind="ExternalOutput")

    with tile.TileContext(nc) as tc:
        with (
            tc.tile_pool(name="singles", bufs=1) as singles,
            tc.tile_pool(name="spool", bufs=int(os.environ.get("K_SBUFS", "2"))) as spool,
            tc.tile_pool(name="temps", bufs=int(os.environ.get("K_TBUFS", "1"))) as temps,
            tc.tile_pool(name="rcpool", bufs=int(os.environ.get("K_RBUFS", "2"))) as rcpool,
            tc.tile_pool(name="wpool", bufs=int(os.environ.get("K_WBUFS", "8"))) as wpool,
            tc.tile_pool(name="tail", bufs=2) as tailp,
            tc.tile_pool(name="psum", bufs=1, space="PSUM") as psum,
        ):
            # ---- one-time loads (split SP/ACT so the loop starts sooner) --
            par = {}
            for name in par_d:
                t = singles.tile([128, CH, N], F32, tag=f"par_{name}")
                eng = nc.sync if name in ("dA_re", "dA_im", "P", "Qn") else nc.scalar
                eng.dma_start(out=t, in_=par_d[name][:, :, :])
                par[name] = t
            u_sb = singles.tile([128, CH, BC], F32, tag="u_sb")
            nc.scalar.dma_start(out=u_sb, in_=u_t_d[:, :, :])
            dp_sb = singles.tile([128, CH], F32, tag="dp_sb")
            nc.scalar.dma_start(out=dp_sb, in_=dp_d[:, :])
            bias_sb = singles.tile([1, J], F32, tag="bias_sb")
            nc.scalar.dma_start(out=bias_sb, in_=bias_d[:, :])
            ones_sb = singles.tile([1, 128], F32, tag="ones_sb")
            nc.vector.memset(ones_sb, 1.0)
            y_all = singles.tile([128, CH, BC], F32, tag="y_all")
            g_all = singles.tile([128, CH, BC], F32, tag="g_all")

            def bcast(t3, cc):
                # [128, CH, N] param slice -> [128, BHS, N] free-broadcast AP
                return t3[:, cc : cc + 1, :].broadcast_to([128, BHS, N])

            # ---- main loop: 8 h-chunks x 2 b-halves -----------------------
            for cc in range(CH):
                hsl = slice(cc * 128, (cc + 1) * 128)
                for hh in range(BH):
                    bsl = slice(hh * BHS, (hh + 1) * BHS)
                    s_re = spool.tile([128, BHS, N], F32, tag="s_re")
                    s_im = spool.tile([128, BHS, N], F32, tag="s_im")
                    nc.sync.dma_start(out=s_re, in_=s_re_d[hsl, bsl, :])
                    nc.sync.dma_start(out=s_im, in_=s_im_d[hsl, bsl, :])

                    ue = u_sb[:, cc, bsl].unsqueeze(2).broadcast_to([128, BHS, N])

                    ta = temps.tile([128, BHS, N], F32, tag="ta")
                    tb = temps.tile([128, BHS, N], F32, tag="tb")
                    tc_ = temps.tile([128, BHS, N], F32, tag="tc")
                    td = temps.tile([128, BHS, N], F32, tag="td")
                    rc2 = rcpool.tile([128, 2, BHS, N], F32, tag="rc2")

                    M = mybir.AluOpType.mult
                    A = mybir.AluOpType.add
                    S = mybir.AluOpType.subtract
                    v, g = nc.vector, nc.gpsimd

                    # DVE: 4 TT + the reduce; Pool: 8 TT (model-balanced).
                    # Broadcast-AP-heavy ops lean on DVE (hardware address
                    # gen); the Q7/Pool side keeps the dense adds.
                    v.tensor_tensor(out=ta, in0=s_re, in1=bcast(par["dA_re"], cc), op=M)
                    g.tensor_tensor(out=tb, in0=s_im, in1=bcast(par["dA_im"], cc), op=M)
                    g.tensor_tensor(out=rc2[:, 0], in0=s_re, in1=bcast(par["P"], cc), op=M)
                    g.tensor_tensor(out=rc2[:, 1], in0=s_im, in1=bcast(par["Qn"], cc), op=M)
                    v.tensor_tensor(out=tc_, in0=s_re, in1=bcast(par["dA_im"], cc), op=M)
                    sp = BHS - int(os.environ.get("K_SPLIT", "8"))
                    dAre_b = bcast(par["dA_re"], cc)
                    g.tensor_tensor(out=td[:, :sp], in0=s_im[:, :sp], in1=dAre_b[:, :sp], op=M)
                    if sp < BHS:
                        v.tensor_tensor(out=td[:, sp:], in0=s_im[:, sp:], in1=dAre_b[:, sp:], op=M)
                    # u terms overwrite the (now dead) state tiles
                    v.tensor_tensor(out=s_re, in0=ue, in1=bcast(par["dB_re"], cc), op=M)
                    g.tensor_tensor(out=ta, in0=ta, in1=tb, op=S)
                    g.tensor_tensor(out=s_re, in0=ta, in1=s_re, op=A)   # = ns_re
                    v.tensor_tensor(out=s_im, in0=ue, in1=bcast(par["dB_im"], cc), op=M)
                    g.tensor_tensor(out=tc_, in0=tc_, in1=td, op=A)
                    g.tensor_tensor(out=s_im, in0=tc_, in1=s_im, op=A)  # = ns_im
                    # y1 half: sum over (component, n)
                    rc2v = rc2.rearrange("p t b n -> p b t n")
                    v.tensor_reduce(
                        out=y_all[:, cc, bsl],
                        in_=rc2v,
                        axis=mybir.AxisListType.XY,
                        op=A,
                    )

                    nc.scalar.dma_start(out=ns_re_d[hsl, bsl, :], in_=s_re)
                    nc.scalar.dma_start(out=ns_im_d[hsl, bsl, :], in_=s_im)

                if os.environ.get("K_STAGE", "full") == "loop":
                    continue
                # tail: g = gelu(y1 + u * Dp) for this chunk
                ty = tailp.tile([128, BC], F32, tag="ty")
                nc.vector.scalar_tensor_tensor(
                    out=ty,
                    in0=u_sb[:, cc, :],
                    scalar=dp_sb[:, cc : cc + 1],
                    in1=y_all[:, cc, :],
                    op0=mybir.AluOpType.mult,
                    op1=mybir.AluOpType.add,
                )
                nc.scalar.activation(
                    out=g_all[:, cc, :],
                    in_=ty,
                    func=mybir.ActivationFunctionType.Gelu,
                )

            # ---- MLP: z = g.T @ W_T + b, GLU ------------------------------
            NF = int(os.environ.get("K_NF", "8"))
            FT = J // NF
            y_out_sb = singles.tile([128, H], F32, tag="y_out_sb")
            stage = os.environ.get("K_STAGE", "full")
            if stage in ("loop", "tail"):
                nc.vector.memset(y_out_sb, 0.0)
                nc.sync.dma_start(out=y_out_d[:, :], in_=y_out_sb)
                stage_skip = True
            else:
                stage_skip = False
            z_tiles = []
            for f in range(NF if not stage_skip else 0):
                w_f = wpool.tile([128, CH, FT], F32, tag="w")
                nc.sync.dma_start(out=w_f, in_=w_d[:, :, f * FT : (f + 1) * FT])
                z_f = psum.tile([128, FT], F32, tag=f"z_{f}")
                z_tiles.append(z_f)
                nc.tensor.matmul(
                    z_f,
                    lhsT=ones_sb,
                    rhs=bias_sb[:, f * FT : (f + 1) * FT],
                    start=True,
                    stop=False,
                )
                for cc in range(CH):
                    nc.tensor.matmul(
                        z_f,
                        lhsT=g_all[:, cc, :],
                        rhs=w_f[:, cc, :],
                        start=False,
                        stop=(cc == CH - 1),
                    )
                if f >= NF // 2:
                    # sigmoid(z2) then z1 * sig
                    fh = f - NF // 2
                    sig = tailp.tile([128, FT], F32, tag="sig")
                    nc.scalar.activation(
                        out=sig, in_=z_f, func=mybir.ActivationFunctionType.Sigmoid
                    )
                    nc.vector.tensor_tensor(
                        out=y_out_sb[:, fh * FT : (fh + 1) * FT],
                        in0=z_tiles[fh],
                        in1=sig,
                        op=mybir.AluOpType.mult,
                    )
            if not stage_skip:
                nc.sync.dma_start(out=y_out_d[:, :], in_=y_out_sb)

    nc.compile()
    return nc


def _get_program():
    if "nc" not in _CACHE:
        _CACHE["nc"] = _build_program()
    return _CACHE["nc"]


def kernel(u, state_re, state_im, dA_re, dA_im, dB_re, dB_im, C_re, C_im, D, W, b):
    u = np.asarray(u, np.float32)
    state_re = np.asarray(state_re, np.float32)
    state_im = np.asarray(state_im, np.float32)
    dA_re = np.asarray(dA_re, np.float32)
    dA_im = np.asarray(dA_im, np.float32)
    dB_re = np.asarray(dB_re, np.float32)
    dB_im = np.asarray(dB_im, np.float32)
    C_re = np.asarray(C_re, np.float32)
    C_im = np.asarray(C_im, np.float32)
    D = np.asarray(D, np.float32)
    W = np.asarray(W, np.float32)
    b = np.asarray(b, np.float32)

    # Host-side parameter folding (see module docstring).
    P = 2.0 * (C_re * dA_re - C_im * dA_im)
    Qn = -2.0 * (C_re * dA_im + C_im * dA_re)
    R = 2.0 * np.sum(C_re * dB_re - C_im * dB_im, axis=1)
    Dp = D + R

    def pmaj(x):  # [H, N] -> [128, CH, N] partition-major
        return np.ascontiguousarray(x.reshape(CH, 128, -1).transpose(1, 0, 2))

    shared = {
        "dA_re": pmaj(dA_re),
        "dA_im": pmaj(dA_im),
        "dB_re": pmaj(dB_re),
        "dB_im": pmaj(dB_im),
        "P": pmaj(P),
        "Qn": pmaj(Qn),
        "Dp": np.ascontiguousarray(Dp.reshape(CH, 128).T),
        "w_t": np.ascontiguousarray(
            W.T.reshape(CH, 128, J).transpose(1, 0, 2)
        ),
        "bias": b.reshape(1, J),
    }

    def core_inputs(m):
        bsl = slice(m * BC, (m + 1) * BC)
        return {
            "s_re": np.ascontiguousarray(state_re[bsl].transpose(1, 0, 2)),
            "s_im": np.ascontiguousarray(state_im[bsl].transpose(1, 0, 2)),
            "u_t": np.ascontiguousarray(
                u[bsl].T.reshape(CH, 128, BC).transpose(1, 0, 2)
            ),
            **shared,
        }

    with ThreadPoolExecutor(NCORES) as ex:
        in_maps = list(ex.map(core_inputs, range(NCORES)))

    nc = _get_program()
    kw = {}
    if os.environ.get("K_TRACE") == "1":
        kw = dict(trace=True, tmpdir=os.environ.get("K_TRACE_DIR") or None)
    res = run_bass_kernel_spmd(nc, in_maps, core_ids=list(range(NCORES)), **kw)
    if kw:
        print(f"HW exec time: {res.exec_time_ns} ns", flush=True)
        _CACHE["last_profile"] = res

    y_out = np.empty((B, H), np.float32)
    ns_re = np.empty((B, H, N), np.float32)
    ns_im = np.empty((B, H, N), np.float32)

    def unpack(m):
        bsl = slice(m * BC, (m + 1) * BC)
        r = res.results[m]
        y_out[bsl] = r["y_out"]
        ns_re[bsl] = r["ns_re"].transpose(1, 0, 2)
        ns_im[bsl] = r["ns_im"].transpose(1, 0, 2)

    with ThreadPoolExecutor(NCORES) as ex:
        list(ex.map(unpack, range(NCORES)))

    return y_out, ns_re, ns_im


# revision 31
# speedup vs baseline: 1.0163x; 1.0163x over previous
"""MetaS4 step kernel for Trainium2, 8 NeuronCores, batch-parallel.

Model (per batch row b, channel h, state index n):
    ns_re = dA_re*s_re - dA_im*s_im + dB_re*u
    ns_im = dA_re*s_im + dA_im*s_re + dB_im*u
    y1    = 2*sum_n(C_re*ns_re - C_im*ns_im)        -- folded to P/Q form below
    y     = gelu(y1 + u*D)
    z     = y @ W.T + b ; out = z[:, :H] * sigmoid(z[:, H:])

Readout is algebraically folded so it reads the OLD state directly:
    y1 = sum_n(P*s_re + Qn*s_im) + u*R
    P  = 2*(C_re*dA_re - C_im*dA_im)
    Qn = -2*(C_re*dA_im + C_im*dA_re)
    R  = 2*sum_n(C_re*dB_re - C_im*dB_im);  Dp = D + R
so gelu input is y1_s + u*Dp, decoupling the readout from the ns chain.

Layout: batch is sharded 128 rows/core.  On-chip, channels h live on the
128 partitions (8 chunks of 128), free dims are (b, n).  All parameters
are per-partition [128, chunk] tables broadcast along free dims via
stride-0 APs, so no parameter materialization is needed.
"""

import os
import sys
from concurrent.futures import ThreadPoolExecutor

import numpy as np

for _p in ("/opt/trn_rl_repo",):
    if _p not in sys.path and os.path.isdir(_p):
        sys.path.insert(0, _p)

import concourse.bass as bass
import concourse.tile as tile
from concourse import bacc, mybir
from concourse.bass_utils import run_bass_kernel_spmd

B, H, N = 1024, 1024, 32
NCORES = 8
BC = B // NCORES          # batch rows per core = 128
CH = 8                    # h chunks of 128 partitions
BH = 2                    # b halves per chunk
BHS = BC // BH            # 64 rows per half
F32 = mybir.dt.float32
J = 2 * H                 # MLP output width

_CACHE = {}


def _build_program():
    nc = bacc.Bacc(
        "TRN2",
        target_bir_lowering=False,
        debug=False,
        enable_asserts=False,
        num_devices=NCORES,
    )

    # Per-core tensors ([H, BC, N] is state transposed so the h-partition
    # DMA is contiguous per partition).
    s_re_d = nc.dram_tensor("s_re", [H, BC, N], F32, kind="ExternalInput")
    s_im_d = nc.dram_tensor("s_im", [H, BC, N], F32, kind="ExternalInput")
    u_t_d = nc.dram_tensor("u_t", [128, CH, BC], F32, kind="ExternalInput")
    # Replicated parameter tables, host-prearranged to partition-major.
    par_d = {
        name: nc.dram_tensor(name, [128, CH, N], F32, kind="ExternalInput")
        for name in ("dA_re", "dA_im", "dB_re", "dB_im", "P", "Qn")
    }
    dp_d = nc.dram_tensor("Dp", [128, CH], F32, kind="ExternalInput")
    w_d = nc.dram_tensor("w_t", [128, CH, J], F32, kind="ExternalInput")
    bias_d = nc.dram_tensor("bias", [1, J], F32, kind="ExternalInput")

    ns_re_d = nc.dram_tensor("ns_re", [H, BC, N], F32, kind="ExternalOutput")
    ns_im_d = nc.dram_tensor("ns_im", [H, BC, N], F32, kind="ExternalOutput")
    y_out_d = nc.dram_tensor("y_out", [BC, H], F32, kind="ExternalOutput")

    with tile.TileContext(nc) as tc:
        with (
            tc.tile_pool(name="singles", bufs=1) as singles,
            tc.tile_pool(name="spool", bufs=int(os.environ.get("K_SBUFS", "2"))) as spool,
            tc.tile_pool(name="temps", bufs=int(os.environ.get("K_TBUFS", "1"))) as temps,
            tc.tile_pool(name="rcpool", bufs=int(os.environ.get("K_RBUFS", "2"))) as rcpool,
            tc.tile_pool(name="wpool", bufs=int(os.environ.get("K_WBUFS", "8"))) as wpool,
            tc.tile_pool(name="tail", bufs=2) as tailp,
            tc.tile_pool(name="psum", bufs=1, space="PSUM") as psum,
        ):
            # ---- one-time loads (split SP/ACT so the loop starts sooner) --
            par = {}
            for name in par_d:
                t = singles.tile([128, CH, N], F32, tag=f"par_{name}")
                eng = nc.sync if name in ("dA_re", "dA_im", "P", "Qn") else nc.scalar
                eng.dma_start(out=t, in_=par_d[name][:, :, :])
                par[name] = t
            u_sb = singles.tile([128, CH, BC], F32, tag="u_sb")
            nc.scalar.dma_start(out=u_sb, in_=u_t_d[:, :, :])
            dp_sb = singles.tile([128, CH], F32, tag="dp_sb")
            nc.scalar.dma_start(out=dp_sb, in_=dp_d[:, :])
            bias_sb = singles.tile([1, J], F32, tag="bias_sb")
            nc.scalar.dma_start(out=bias_sb, in_=bias_d[:, :])
            ones_sb = singles.tile([1, 128], F32, tag="ones_sb")
            nc.vector.memset(ones_sb, 1.0)
            y_all = singles.tile([128, CH, BC], F32, tag="y_all")
            g_all = singles.tile([128, CH, BC], F32, tag="g_all")

            def bcast(t3, cc):
                # [128, CH, N] param slice -> [128, BHS, N] free-broadcast AP
                return t3[:, cc : cc + 1, :].broadcast_to([128, BHS, N])

            # ---- main loop: 8 h-chunks x 2 b-halves -----------------------
            for cc in range(CH):
                hsl = slice(cc * 128, (cc + 1) * 128)
                for hh in range(BH):
                    bsl = slice(hh * BHS, (hh + 1) * BHS)
                    s_re = spool.tile([128, BHS, N], F32, tag="s_re")
                    s_im = spool.tile([128, BHS, N], F32, tag="s_im")
                    nc.sync.dma_start(out=s_re, in_=s_re_d[hsl, bsl, :])
                    nc.sync.dma_start(out=s_im, in_=s_im_d[hsl, bsl, :])

                    ue = u_sb[:, cc, bsl].unsqueeze(2).broadcast_to([128, BHS, N])

                    ta = temps.tile([128, BHS, N], F32, tag="ta")
                    tb = temps.tile([128, BHS, N], F32, tag="tb")
                    tc_ = temps.tile([128, BHS, N], F32, tag="tc")
                    td = temps.tile([128, BHS, N], F32, tag="td")
                    rc2 = rcpool.tile([128, 2, BHS, N], F32, tag="rc2")

                    M = mybir.AluOpType.mult
                    A = mybir.AluOpType.add
                    S = mybir.AluOpType.subtract
                    v, g = nc.vector, nc.gpsimd

                    # DVE: 4 TT + the reduce; Pool: 8 TT (model-balanced)
                    v.tensor_tensor(out=ta, in0=s_re, in1=bcast(par["dA_re"], cc), op=M)
                    g.tensor_tensor(out=tb, in0=s_im, in1=bcast(par["dA_im"], cc), op=M)
                    v.tensor_tensor(out=rc2[:, 0], in0=s_re, in1=bcast(par["P"], cc), op=M)
                    g.tensor_tensor(out=rc2[:, 1], in0=s_im, in1=bcast(par["Qn"], cc), op=M)
                    v.tensor_tensor(out=tc_, in0=s_re, in1=bcast(par["dA_im"], cc), op=M)
                    sp = BHS - int(os.environ.get("K_SPLIT", "8"))
                    dAre_b = bcast(par["dA_re"], cc)
                    g.tensor_tensor(out=td[:, :sp], in0=s_im[:, :sp], in1=dAre_b[:, :sp], op=M)
                    if sp < BHS:
                        v.tensor_tensor(out=td[:, sp:], in0=s_im[:, sp:], in1=dAre_b[:, sp:], op=M)
                    # u terms overwrite the (now dead) state tiles
                    g.tensor_tensor(out=s_re, in0=ue, in1=bcast(par["dB_re"], cc), op=M)
                    v.tensor_tensor(out=ta, in0=ta, in1=tb, op=S)
                    g.tensor_tensor(out=s_re, in0=ta, in1=s_re, op=A)   # = ns_re
                    g.tensor_tensor(out=s_im, in0=ue, in1=bcast(par["dB_im"], cc), op=M)
                    g.tensor_tensor(out=tc_, in0=tc_, in1=td, op=A)
                    g.tensor_tensor(out=s_im, in0=tc_, in1=s_im, op=A)  # = ns_im
                    # y1 half: sum over (component, n)
                    rc2v = rc2.rearrange("p t b n -> p b t n")
                    v.tensor_reduce(
                        out=y_all[:, cc, bsl],
                        in_=rc2v,
                        axis=mybir.AxisListType.XY,
                        op=A,
                    )

                    nc.scalar.dma_start(out=ns_re_d[hsl, bsl, :], in_=s_re)
                    nc.scalar.dma_start(out=ns_im_d[hsl, bsl, :], in_=s_im)

                if os.environ.get("K_STAGE", "full") == "loop":
                    continue
                # tail: g = gelu(y1 + u * Dp) for this chunk
                ty = tailp.tile([128, BC], F32, tag="ty")
                nc.vector.scalar_tensor_tensor(
                    out=ty,
                    in0=u_sb[:, cc, :],
                    scalar=dp_sb[:, cc : cc + 1],
                    in1=y_all[:, cc, :],
                    op0=mybir.AluOpType.mult,
                    op1=mybir.AluOpType.add,
                )
                nc.scalar.activation(
                    out=g_all[:, cc, :],
                    in_=ty,
                    func=mybir.ActivationFunctionType.Gelu,
                )

            # ---- MLP: z = g.T @ W_T + b, GLU ------------------------------
            NF = int(os.environ.get("K_NF", "8"))
            FT = J // NF
            y_out_sb = singles.tile([128, H], F32, tag="y_out_sb")
            stage = os.environ.get("K_STAGE", "full")
            if stage in ("loop", "tail"):
                nc.vector.memset(y_out_sb, 0.0)
                nc.sync.dma_start(out=y_out_d[:, :], in_=y_out_sb)
                stage_skip = True
            else:
                stage_skip = False
            z_tiles = []
            for f in range(NF if not stage_skip else 0):
                w_f = wpool.tile([128, CH, FT], F32, tag="w")
                nc.sync.dma_start(out=w_f, in_=w_d[:, :, f * FT : (f + 1) * FT])
                z_f = psum.tile([128, FT], F32, tag=f"z_{f}")
                z_tiles.append(z_f)
                nc.tensor.matmul(
                    z_f,
                    lhsT=ones_sb,
                    rhs=bias_sb[:, f * FT : (f + 1) * FT],
                    start=True,
                    stop=False,
                )
                for cc in range(CH):
                    nc.tensor.matmul(
                        z_f,
                        lhsT=g_all[:, cc, :],
                        rhs=w_f[:, cc, :],
                        start=False,
                        stop=(cc == CH - 1),
                    )
                if f >= NF // 2:
                    # sigmoid(z2) then z1 * sig
                    fh = f - NF // 2
                    sig = tailp.tile([128, FT], F32, tag="sig")
                    nc.scalar.activation(
                        out=sig, in_=z_f, func=mybir.ActivationFunctionType.Sigmoid
                    )
                    nc.vector.tensor_tensor(
                        out=y_out_sb[:, fh * FT : (fh + 1) * FT],
                        in0=z_tiles[fh],
                        in1=sig,
                        op=mybir.AluOpType.mult,
                    )
            if not stage_skip:
                nc.sync.dma_start(out=y_out_d[:, :], in_=y_out_sb)

    nc.compile()
    return nc


def _get_program():
    if "nc" not in _CACHE:
        _CACHE["nc"] = _build_program()
    return _CACHE["nc"]


def kernel(u, state_re, state_im, dA_re, dA_im, dB_re, dB_im, C_re, C_im, D, W, b):
    u = np.asarray(u, np.float32)
    state_re = np.asarray(state_re, np.float32)
    state_im = np.asarray(state_im, np.float32)
    dA_re = np.asarray(dA_re, np.float32)
    dA_im = np.asarray(dA_im, np.float32)
    dB_re = np.asarray(dB_re, np.float32)
    dB_im = np.asarray(dB_im, np.float32)
    C_re = np.asarray(C_re, np.float32)
    C_im = np.asarray(C_im, np.float32)
    D = np.asarray(D, np.float32)
    W = np.asarray(W, np.float32)
    b = np.asarray(b, np.float32)

    # Host-side parameter folding (see module docstring).
    P = 2.0 * (C_re * dA_re - C_im * dA_im)
    Qn = -2.0 * (C_re * dA_im + C_im * dA_re)
    R = 2.0 * np.sum(C_re * dB_re - C_im * dB_im, axis=1)
    Dp = D + R

    def pmaj(x):  # [H, N] -> [128, CH, N] partition-major
        return np.ascontiguousarray(x.reshape(CH, 128, -1).transpose(1, 0, 2))

    shared = {
        "dA_re": pmaj(dA_re),
        "dA_im": pmaj(dA_im),
        "dB_re": pmaj(dB_re),
        "dB_im": pmaj(dB_im),
        "P": pmaj(P),
        "Qn": pmaj(Qn),
        "Dp": np.ascontiguousarray(Dp.reshape(CH, 128).T),
        "w_t": np.ascontiguousarray(
            W.T.reshape(CH, 128, J).transpose(1, 0, 2)
        ),
        "bias": b.reshape(1, J),
    }

    def core_inputs(m):
        bsl = slice(m * BC, (m + 1) * BC)
        return {
            "s_re": np.ascontiguousarray(state_re[bsl].transpose(1, 0, 2)),
            "s_im": np.ascontiguousarray(state_im[bsl].transpose(1, 0, 2)),
            "u_t": np.ascontiguousarray(
                u[bsl].T.reshape(CH, 128, BC).transpose(1, 0, 2)
            ),
            **shared,
        }

    with ThreadPoolExecutor(NCORES) as ex:
        in_maps = list(ex.map(core_inputs, range(NCORES)))

    nc = _get_program()
    kw = {}
    if os.environ.get("K_TRACE") == "1":
        kw = dict(trace=True, tmpdir=os.environ.get("K_TRACE_DIR") or None)
    res = run_bass_kernel_spmd(nc, in_maps, core_ids=list(range(NCORES)), **kw)
    if kw:
        print(f"HW exec time: {res.exec_time_ns} ns", flush=True)
        _CACHE["last_profile"] = res

    y_out = np.empty((B, H), np.float32)
    ns_re = np.empty((B, H, N), np.float32)
    ns_im = np.empty((B, H, N), np.float32)

    def unpack(m):
        bsl = slice(m * BC, (m + 1) * BC)
        r = res.results[m]
        y_out[bsl] = r["y_out"]
        ns_re[bsl] = r["ns_re"].transpose(1, 0, 2)
        ns_im[bsl] = r["ns_im"].transpose(1, 0, 2)

    with ThreadPoolExecutor(NCORES) as ex:
        list(ex.map(unpack, range(NCORES)))

    return y_out, ns_re, ns_im
